# revision 9
# baseline (speedup 1.0000x reference)
"""Trainium2 Bass kernel for nn_Attention_3298534884255.

Computes, for inputs x:[S,B,H], hidden:[1,B,H], pad:[B,S], W,U:[H,H], v:[H,1]:
    scores[s,b] = v . tanh(hidden[0]@W [b] + (x[s,b] @ U))
    out = softmax(where(pad, -1e5, scores.T), axis=1)   -> [B, S]

Strategy: pure data parallelism over batch B=64 across 8 NeuronCores (8 batch
rows per core). W/U/v are tiny and replicated; the heavy op is x@U
(2*S*B*H^2 = 275 GFLOP total). Per core the matmul is computed in a
"proj-transposed" layout: psum[h_out, row] = sum_k U[k,h_out] * xT[k,row],
so U's natural layout is the stationary operand and xT (host-pretransposed)
streams. The Wh bias is per-partition in this layout (rows of one block share
one batch), so it fuses into the scalar-engine tanh. The v-dot is an
accumulating PE matmul over the 8 h_out chunks; softmax runs on-chip.

S, B, H = 2048, 64, 1024. fp16 operands into the PE (fp32 accumulation).
"""

import os
import sys

import numpy as np

if "/opt/trn_rl_repo" not in sys.path:
    sys.path.insert(0, "/opt/trn_rl_repo")

import concourse.bass as bass
import concourse.tile as tile
from concourse import bacc, mybir
from concourse.bass_utils import run_bass_kernel_spmd

S, B, H = 2048, 64, 1024
NCORES = 8
BLOC = B // NCORES          # batch rows per core = 8
ROWS = S * BLOC             # rows per core = 16384
NBLK = 512                  # rows per block (one PSUM bank of fp32)
SBLK = S // NBLK            # s-blocks per batch row = 4
KC = H // 128               # contraction chunks = 8
MC = H // 128               # h_out chunks = 8

F16 = mybir.dt.float16
F32 = mybir.dt.float32


def _build_program(reps=1, vdot="pe"):
    nc = bacc.Bacc(
        "TRN2", target_bir_lowering=False, debug=False, num_devices=NCORES
    )

    xt = nc.dram_tensor("xt", [H, ROWS], F16, kind="ExternalInput").ap()
    ut = nc.dram_tensor("ut", [128, KC * MC * 128], F16, kind="ExternalInput").ap()
    wh = nc.dram_tensor("wh", [128, MC * BLOC], F32, kind="ExternalInput").ap()
    vv = nc.dram_tensor("vv", [128, MC], F16, kind="ExternalInput").ap()
    mask = nc.dram_tensor("mask", [BLOC, S], F32, kind="ExternalInput").ap()
    out = nc.dram_tensor("out", [BLOC, S], F32, kind="ExternalOutput").ap()

    with tile.TileContext(nc) as tc:
        with (
            tc.tile_pool(name="consts", bufs=1) as consts,
            tc.tile_pool(name="xblk", bufs=3) as xpool,
            tc.tile_pool(name="tanh", bufs=4) as tpool,
            tc.tile_pool(name="proj_ps", bufs=3, space="PSUM") as pspool,
            tc.tile_pool(name="score_ps", bufs=2, space="PSUM") as scpool,
            tc.tile_pool(name="softmax", bufs=1) as smpool,
        ):
            u_sb = consts.tile([128, KC * MC * 128], F16)
            nc.sync.dma_start(u_sb[:], ut[:])
            wh_sb = consts.tile([128, MC * BLOC], F32)
            nc.sync.dma_start(wh_sb[:], wh[:])
            v_sb = consts.tile([128, MC], F16)
            nc.sync.dma_start(v_sb[:], vv[:])
            ones_sb = consts.tile([128, 1], F16)
            nc.vector.memset(ones_sb[:], 1.0)
            mask_sb = consts.tile([BLOC, S], F32)
            nc.sync.dma_start(mask_sb[:], mask[:])
            strip = consts.tile([1, ROWS], F32)
            scores_sb = consts.tile([BLOC, S], F32)

            xt_r = xt.rearrange("(k p) n -> p k n", p=128)

            for b, sb in [
                (b, sb)
                for _ in range(reps)
                for b in range(BLOC)
                for sb in range(SBLK)
            ]:
                    g0 = b * S + sb * NBLK
                    xb = xpool.tile([128, KC * NBLK], F16, tag="xb")
                    nc.sync.dma_start(
                        xb[:].rearrange("p (k n) -> p k n", k=KC),
                        xt_r[:, :, g0 : g0 + NBLK],
                    )
                    sc = scpool.tile([1, NBLK], F32, tag="sc")
                    acc = None
                    for m in range(MC):
                        pt = pspool.tile([128, NBLK], F32, tag="pt")
                        for k in range(KC):
                            nc.tensor.matmul(
                                pt[:],
                                u_sb[:, (k * MC + m) * 128 : (k * MC + m + 1) * 128],
                                xb[:, k * NBLK : (k + 1) * NBLK],
                                start=(k == 0),
                                stop=(k == KC - 1),
                            )
                        th = tpool.tile([128, NBLK], F16, tag="th")
                        nc.scalar.activation(
                            th[:],
                            pt[:],
                            mybir.ActivationFunctionType.Tanh,
                            bias=wh_sb[:, m * BLOC + b : m * BLOC + b + 1],
                        )
                        if vdot == "pe":
                            nc.tensor.matmul(
                                sc[:],
                                v_sb[:, m : m + 1],
                                th[:],
                                start=(m == 0),
                                stop=(m == MC - 1),
                            )
                        else:
                            # acc = th * v[m] (+ acc)  on the vector engine
                            if m == 0:
                                acc = tpool.tile([128, NBLK], F16, tag="acc")
                                nc.vector.tensor_scalar_mul(
                                    acc[:], th[:], v_sb[:, m : m + 1]
                                )
                            else:
                                nc.vector.scalar_tensor_tensor(
                                    acc[:],
                                    th[:],
                                    v_sb[:, m : m + 1],
                                    acc[:],
                                    op0=mybir.AluOpType.mult,
                                    op1=mybir.AluOpType.add,
                                )
                    if vdot == "dve":
                        nc.tensor.matmul(
                            sc[:], ones_sb[:], acc[:], start=True, stop=True
                        )
                    nc.scalar.copy(
                        strip[:, g0 : g0 + NBLK], sc[:]
                    )

            # redistribute the [1, ROWS] b-major strip to [BLOC, S]
            nc.sync.dma_start(scores_sb[:], strip[:])

            # softmax over S for each of the BLOC batch rows
            masked = smpool.tile([BLOC, S], F32, tag="masked")
            nc.vector.tensor_add(masked[:], scores_sb[:], mask_sb[:])
            negmax = smpool.tile([BLOC, 1], F32, tag="negmax")
            nc.vector.reduce_max(
                negmax[:], masked[:], axis=mybir.AxisListType.X, negate=True
            )
            probs = smpool.tile([BLOC, S], F32, tag="probs")
            sumexp = smpool.tile([BLOC, 1], F32, tag="sumexp")
            nc.scalar.activation(
                probs[:],
                masked[:],
                mybir.ActivationFunctionType.Exp,
                bias=negmax[:],
                accum_out=sumexp[:],
            )
            rsum = smpool.tile([BLOC, 1], F32, tag="rsum")
            nc.vector.reciprocal(rsum[:], sumexp[:])
            nc.vector.tensor_scalar_mul(probs[:], probs[:], rsum[:])
            nc.sync.dma_start(out[:], probs[:])

    nc.compile()
    return nc


_NC = None


def _get_program():
    global _NC
    if _NC is None:
        _NC = _build_program()
    return _NC


def _prepare_in_maps(inputs, hidden, pad_matrix, W, U, v):
    inputs = np.asarray(inputs, dtype=np.float32)
    hidden = np.asarray(hidden, dtype=np.float32)
    pad_matrix = np.asarray(pad_matrix)
    W = np.asarray(W, dtype=np.float32)
    U = np.asarray(U, dtype=np.float32)
    v = np.asarray(v, dtype=np.float32)

    # xT_all[h, b, s] = inputs[s, b, h]
    xt_all = np.ascontiguousarray(inputs.transpose(2, 1, 0)).astype(np.float16)
    # U tiled: ut[p, ((k*MC + m)*128 + j)] = U[k*128+p, m*128+j]
    ut = np.ascontiguousarray(
        U.reshape(KC, 128, MC, 128).transpose(1, 0, 2, 3)
    ).reshape(128, KC * MC * 128).astype(np.float16)
    # bias Wh = hidden[0] @ W, fp32 on host (0.05% of total FLOPs)
    Wh = hidden[0] @ W  # [B, H]
    # v tiled: vv[p, m] = v[m*128+p]
    vv = np.ascontiguousarray(v[:, 0].reshape(MC, 128).T).astype(np.float16)

    in_maps = []
    for c in range(NCORES):
        b0 = c * BLOC
        xt_c = np.ascontiguousarray(
            xt_all[:, b0 : b0 + BLOC, :].reshape(H, ROWS)
        )
        # wh[p, m*BLOC + b] = Wh[b0+b, m*128+p]
        wh_c = np.ascontiguousarray(
            Wh[b0 : b0 + BLOC].reshape(BLOC, MC, 128).transpose(2, 1, 0)
        ).reshape(128, MC * BLOC).astype(np.float32)
        mask_c = np.where(pad_matrix[b0 : b0 + BLOC], -100000.0, 0.0).astype(
            np.float32
        )
        in_maps.append(
            {"xt": xt_c, "ut": ut, "wh": wh_c, "vv": vv, "mask": mask_c}
        )
    return in_maps


def kernel(inputs, hidden, pad_matrix, W, U, v):
    nc = _get_program()
    in_maps = _prepare_in_maps(inputs, hidden, pad_matrix, W, U, v)
    res = run_bass_kernel_spmd(nc, in_maps, core_ids=list(range(NCORES)))
    out = np.concatenate([r["out"] for r in res.results], axis=0)
    return out.astype(np.float32)


# revision 14
# speedup vs baseline: 1.1350x; 1.1350x over previous
"""Trainium2 Bass kernel for nn_Attention_3298534884255.

Computes, for inputs x:[S,B,H], hidden:[1,B,H], pad:[B,S], W,U:[H,H], v:[H,1]:
    scores[s,b] = v . tanh(hidden[0]@W [b] + (x[s,b] @ U))
    out = softmax(where(pad, -1e5, scores.T), axis=1)   -> [B, S]

Strategy: pure data parallelism over batch B=64 across 8 NeuronCores (8 batch
rows per core). W/U/v are tiny and replicated; the heavy op is x@U
(2*S*B*H^2 = 275 GFLOP total). Per core the matmul is computed in a
"proj-transposed" layout: psum[h_out, row] = sum_k U[k,h_out] * xT[k,row],
so U's natural layout is the stationary operand and xT (host-pretransposed)
streams. The Wh bias is per-partition in this layout (rows of one block share
one batch), so it fuses into the scalar-engine tanh. The v-dot is an
accumulating PE matmul over the 8 h_out chunks; softmax runs on-chip.

S, B, H = 2048, 64, 1024. fp16 operands into the PE (fp32 accumulation).
"""

import os
import sys

import numpy as np

if "/opt/trn_rl_repo" not in sys.path:
    sys.path.insert(0, "/opt/trn_rl_repo")

import concourse.bass as bass
import concourse.tile as tile
from concourse import bacc, mybir
from concourse.bass_utils import run_bass_kernel_spmd

S, B, H = 2048, 64, 1024
NCORES = 8
BLOC = B // NCORES          # batch rows per core = 8
ROWS = S * BLOC             # rows per core = 16384
NBLK = 512                  # rows per block (one PSUM bank of fp32)
SBLK = S // NBLK            # s-blocks per batch row = 4
KC = H // 128               # contraction chunks = 8
MC = H // 128               # h_out chunks = 8

F16 = mybir.dt.float16
F32 = mybir.dt.float32


def _build_program(reps=1, vdot="pe", nblk=NBLK):
    sblk = S // nblk
    nc = bacc.Bacc(
        "TRN2", target_bir_lowering=False, debug=False, num_devices=NCORES
    )

    xt = nc.dram_tensor("xt", [H, ROWS], F16, kind="ExternalInput").ap()
    ut = nc.dram_tensor("ut", [128, KC * MC * 128], F16, kind="ExternalInput").ap()
    wh = nc.dram_tensor("wh", [128, MC * BLOC], F32, kind="ExternalInput").ap()
    vv = nc.dram_tensor("vv", [128, MC], F16, kind="ExternalInput").ap()
    vvf = nc.dram_tensor("vvf", [128, MC], F32, kind="ExternalInput").ap()
    mask = nc.dram_tensor("mask", [BLOC, S], F32, kind="ExternalInput").ap()
    out = nc.dram_tensor("out", [BLOC, S], F32, kind="ExternalOutput").ap()

    with tile.TileContext(nc) as tc:
        with (
            tc.tile_pool(name="consts", bufs=1) as consts,
            tc.tile_pool(name="xblk", bufs=3) as xpool,
            tc.tile_pool(name="tanh", bufs=4) as tpool,
            tc.tile_pool(name="proj_ps", bufs=3, space="PSUM") as pspool,
            tc.tile_pool(name="score_ps", bufs=2, space="PSUM") as scpool,
            tc.tile_pool(name="softmax", bufs=1) as smpool,
        ):
            u_sb = consts.tile([128, KC * MC * 128], F16)
            nc.sync.dma_start(u_sb[:], ut[:])
            wh_sb = consts.tile([128, MC * BLOC], F32)
            nc.sync.dma_start(wh_sb[:], wh[:])
            v_sb = consts.tile([128, MC], F16)
            nc.sync.dma_start(v_sb[:], vv[:])
            v32_sb = consts.tile([128, MC], F32)
            nc.sync.dma_start(v32_sb[:], vvf[:])
            ones_sb = consts.tile([128, 1], F16)
            nc.vector.memset(ones_sb[:], 1.0)
            mask_sb = consts.tile([BLOC, S], F32)
            nc.sync.dma_start(mask_sb[:], mask[:])
            strip = consts.tile([1, ROWS], F32)
            scores_sb = consts.tile([BLOC, S], F32)

            xt_r = xt.rearrange("(k p) n -> p k n", p=128)

            for b, sb in [
                (b, sb)
                for _ in range(reps)
                for b in range(BLOC)
                for sb in range(sblk)
            ]:
                    g0 = b * S + sb * nblk
                    xb = xpool.tile([128, KC * nblk], F16, tag="xb")
                    nc.sync.dma_start(
                        xb[:].rearrange("p (k n) -> p k n", k=KC),
                        xt_r[:, :, g0 : g0 + nblk],
                    )
                    sc = scpool.tile([1, nblk], F32, tag="sc")
                    acc = None
                    for m in range(MC):
                        pt = pspool.tile([128, nblk], F32, tag="pt")
                        for k in range(KC):
                            nc.tensor.matmul(
                                pt[:],
                                u_sb[:, (k * MC + m) * 128 : (k * MC + m + 1) * 128],
                                xb[:, k * nblk : (k + 1) * nblk],
                                start=(k == 0),
                                stop=(k == KC - 1),
                            )
                        th = tpool.tile([128, nblk], F16, tag="th")
                        nc.scalar.activation(
                            th[:],
                            pt[:],
                            mybir.ActivationFunctionType.Tanh,
                            bias=wh_sb[:, m * BLOC + b : m * BLOC + b + 1],
                        )
                        if vdot == "pe":
                            nc.tensor.matmul(
                                sc[:],
                                v_sb[:, m : m + 1],
                                th[:],
                                start=(m == 0),
                                stop=(m == MC - 1),
                            )
                        else:
                            # acc = th * v[m] (+ acc)  on the vector engine
                            if m == 0:
                                acc = tpool.tile([128, nblk], F16, tag="acc")
                                nc.vector.tensor_scalar_mul(
                                    acc[:], th[:], v_sb[:, m : m + 1]
                                )
                            else:
                                nc.vector.scalar_tensor_tensor(
                                    acc[:],
                                    th[:],
                                    v_sb[:, m : m + 1],
                                    acc[:],
                                    op0=mybir.AluOpType.mult,
                                    op1=mybir.AluOpType.add,
                                )
                    if vdot == "dve":
                        nc.tensor.matmul(
                            sc[:], ones_sb[:], acc[:], start=True, stop=True
                        )
                    nc.scalar.copy(
                        strip[:, g0 : g0 + nblk], sc[:]
                    )

            # redistribute the [1, ROWS] b-major strip to [BLOC, S]
            nc.sync.dma_start(scores_sb[:], strip[:])

            # softmax over S for each of the BLOC batch rows
            masked = smpool.tile([BLOC, S], F32, tag="masked")
            nc.vector.tensor_add(masked[:], scores_sb[:], mask_sb[:])
            negmax = smpool.tile([BLOC, 1], F32, tag="negmax")
            nc.vector.reduce_max(
                negmax[:], masked[:], axis=mybir.AxisListType.X, negate=True
            )
            probs = smpool.tile([BLOC, S], F32, tag="probs")
            sumexp = smpool.tile([BLOC, 1], F32, tag="sumexp")
            nc.scalar.activation(
                probs[:],
                masked[:],
                mybir.ActivationFunctionType.Exp,
                bias=negmax[:],
                accum_out=sumexp[:],
            )
            rsum = smpool.tile([BLOC, 1], F32, tag="rsum")
            nc.vector.reciprocal(rsum[:], sumexp[:])
            nc.vector.tensor_scalar_mul(probs[:], probs[:], rsum[:])
            nc.sync.dma_start(out[:], probs[:])

    nc.compile()
    return nc


_NC = None


def _get_program():
    global _NC
    if _NC is None:
        _NC = _build_program()
    return _NC


def _prepare_in_maps(inputs, hidden, pad_matrix, W, U, v):
    inputs = np.asarray(inputs, dtype=np.float32)
    hidden = np.asarray(hidden, dtype=np.float32)
    pad_matrix = np.asarray(pad_matrix)
    W = np.asarray(W, dtype=np.float32)
    U = np.asarray(U, dtype=np.float32)
    v = np.asarray(v, dtype=np.float32)

    # xT_all[h, b, s] = inputs[s, b, h]
    xt_all = np.ascontiguousarray(inputs.transpose(2, 1, 0)).astype(np.float16)
    # U tiled: ut[p, ((k*MC + m)*128 + j)] = U[k*128+p, m*128+j]
    ut = np.ascontiguousarray(
        U.reshape(KC, 128, MC, 128).transpose(1, 0, 2, 3)
    ).reshape(128, KC * MC * 128).astype(np.float16)
    # bias Wh = hidden[0] @ W, fp32 on host (0.05% of total FLOPs)
    Wh = hidden[0] @ W  # [B, H]
    # v tiled: vv[p, m] = v[m*128+p]
    vvf = np.ascontiguousarray(v[:, 0].reshape(MC, 128).T).astype(np.float32)
    vv = vvf.astype(np.float16)

    in_maps = []
    for c in range(NCORES):
        b0 = c * BLOC
        xt_c = np.ascontiguousarray(
            xt_all[:, b0 : b0 + BLOC, :].reshape(H, ROWS)
        )
        # wh[p, m*BLOC + b] = Wh[b0+b, m*128+p]
        wh_c = np.ascontiguousarray(
            Wh[b0 : b0 + BLOC].reshape(BLOC, MC, 128).transpose(2, 1, 0)
        ).reshape(128, MC * BLOC).astype(np.float32)
        mask_c = np.where(pad_matrix[b0 : b0 + BLOC], -100000.0, 0.0).astype(
            np.float32
        )
        in_maps.append(
            {"xt": xt_c, "ut": ut, "wh": wh_c, "vv": vv, "vvf": vvf,
             "mask": mask_c}
        )
    return in_maps


def kernel(inputs, hidden, pad_matrix, W, U, v):
    nc = _get_program()
    in_maps = _prepare_in_maps(inputs, hidden, pad_matrix, W, U, v)
    res = run_bass_kernel_spmd(nc, in_maps, core_ids=list(range(NCORES)))
    out = np.concatenate([r["out"] for r in res.results], axis=0)
    return out.astype(np.float32)


# revision 30
# speedup vs baseline: 1.3885x; 1.2234x over previous
"""Trainium2 Bass kernel for nn_Attention_3298534884255.

Computes, for inputs x:[S,B,H], hidden:[1,B,H], pad:[B,S], W,U:[H,H], v:[H,1]:
    scores[s,b] = v . tanh(hidden[0]@W [b] + (x[s,b] @ U))
    out = softmax(where(pad, -1e5, scores.T), axis=1)   -> [B, S]

Strategy: pure data parallelism over batch B=64 across 8 NeuronCores (8 batch
rows per core). W/U/v are tiny and replicated; the heavy op is x@U
(2*S*B*H^2 = 275 GFLOP total). Per core the matmul is computed in a
"proj-transposed" layout: psum[h_out, row] = sum_k U[k,h_out] * xT[k,row],
so U's natural layout is the stationary operand and xT (host-pretransposed)
streams. The Wh bias is per-partition in this layout (rows of one block share
one batch), so it fuses into the scalar-engine tanh. The v-dot runs on the
vector engine (scalar_tensor_tensor chain over the 8 h_out chunks) plus one
ones-vector matmul for the final partition reduction, keeping the PE free for
the main matmul stream (~220ns per 128x128x512 fp16 matmul = streaming
roofline). Per-batch softmax overlaps the remaining compute; only the last
row's softmax sits in the kernel tail.

S, B, H = 2048, 64, 1024. fp16 operands into the PE (fp32 accumulation).
"""

import os
import sys

import numpy as np

if "/opt/trn_rl_repo" not in sys.path:
    sys.path.insert(0, "/opt/trn_rl_repo")

import concourse.bass as bass
import concourse.tile as tile
from concourse import bacc, mybir
from concourse.bass_utils import run_bass_kernel_spmd

S, B, H = 2048, 64, 1024
NCORES = 8
BLOC = B // NCORES          # batch rows per core = 8
ROWS = S * BLOC             # rows per core = 16384
NBLK = 512                  # rows per block (one PSUM bank of fp32)
SBLK = S // NBLK            # s-blocks per batch row = 4
KC = H // 128               # contraction chunks = 8
MC = H // 128               # h_out chunks = 8

F16 = mybir.dt.float16
F32 = mybir.dt.float32


def _build_program(reps=1, vdot="dve", nblk=NBLK):
    sblk = S // nblk
    nc = bacc.Bacc(
        "TRN2", target_bir_lowering=False, debug=False, num_devices=NCORES
    )

    xt = nc.dram_tensor("xt", [H, ROWS], F16, kind="ExternalInput").ap()
    ut = nc.dram_tensor("ut", [128, KC * MC * 128], F16, kind="ExternalInput").ap()
    wh = nc.dram_tensor("wh", [128, MC * BLOC], F32, kind="ExternalInput").ap()
    vv = nc.dram_tensor("vv", [128, MC], F16, kind="ExternalInput").ap()
    vvf = nc.dram_tensor("vvf", [128, MC], F32, kind="ExternalInput").ap()
    mask = nc.dram_tensor("mask", [BLOC, S], F32, kind="ExternalInput").ap()
    out = nc.dram_tensor("out", [BLOC, S], F32, kind="ExternalOutput").ap()

    with tile.TileContext(nc) as tc:
        with (
            tc.tile_pool(name="consts", bufs=1) as consts,
            tc.tile_pool(name="xblk", bufs=3) as xpool,
            tc.tile_pool(name="tanh", bufs=4) as tpool,
            tc.tile_pool(
                name="proj_ps", bufs=(3 if nblk <= 512 else 2), space="PSUM"
            ) as pspool,
            tc.tile_pool(name="score_ps", bufs=2, space="PSUM") as scpool,
            tc.tile_pool(name="softmax", bufs=1) as smpool,
        ):
            # U arrives in per-k-chunk DMAs so the first block's matmuls can
            # start as soon as chunk 0 + the first x block land; small consts
            # ride the gpsimd (SWDGE) queue in parallel with the sync queue.
            u_sb = consts.tile([128, KC * MC * 128], F16)
            u_rest_loaded = [False]
            ucw = MC * 128
            nc.sync.dma_start(u_sb[:, 0:ucw], ut[:, 0:ucw])
            wh_sb = consts.tile([128, MC * BLOC], F32)
            nc.gpsimd.dma_start(wh_sb[:], wh[:])
            v_sb = consts.tile([128, MC], F16)
            nc.gpsimd.dma_start(v_sb[:], vv[:])
            v32_sb = consts.tile([128, MC], F32)
            nc.gpsimd.dma_start(v32_sb[:], vvf[:])
            ones_sb = consts.tile([128, 1], F16)
            nc.vector.memset(ones_sb[:], 1.0)
            mask_sb = consts.tile([1, ROWS], F32)
            nc.gpsimd.dma_start(mask_sb[:], mask[:])
            strip = consts.tile([1, ROWS], F32)

            # warm the PE clock (HAM) with throwaway matmuls while the first
            # real operands are still in flight on the DMA queues
            warm_sb = consts.tile([128, 512], F16)
            nc.vector.memset(warm_sb[:], 0.0)
            warm_ps = pspool.tile([128, 512], F32, tag="pt")
            for _ in range(28):
                nc.tensor.matmul(
                    warm_ps[:], warm_sb[:, 0:128], warm_sb[:],
                    start=True, stop=True,
                )

            xt_r = xt.rearrange("(k p) n -> p k n", p=128)

            negmax = smpool.tile([1, BLOC], F32, tag="negmax")
            sumexp = smpool.tile([1, BLOC], F32, tag="sumexp")
            rsum = smpool.tile([1, BLOC], F32, tag="rsum")

            for b, sb in [
                (b, sb)
                for _ in range(reps)
                for b in range(BLOC)
                for sb in range(sblk)
            ]:
                    g0 = b * S + sb * nblk
                    xb = xpool.tile([128, KC * nblk], F16, tag="xb")
                    nc.sync.dma_start(
                        xb[:].rearrange("p (k n) -> p k n", k=KC),
                        xt_r[:, :, g0 : g0 + nblk],
                    )
                    if not u_rest_loaded[0]:
                        # remaining U chunks, behind the first x block
                        u_rest_loaded[0] = True
                        for kk in range(1, KC):
                            nc.sync.dma_start(
                                u_sb[:, kk * ucw : (kk + 1) * ucw],
                                ut[:, kk * ucw : (kk + 1) * ucw],
                            )
                    sc = scpool.tile([1, nblk], F32, tag="sc")
                    acc = None
                    for m in range(MC):
                        pt = pspool.tile([128, nblk], F32, tag="pt")
                        for k in range(KC):
                            nc.tensor.matmul(
                                pt[:],
                                u_sb[:, (k * MC + m) * 128 : (k * MC + m + 1) * 128],
                                xb[:, k * nblk : (k + 1) * nblk],
                                start=(k == 0),
                                stop=(k == KC - 1),
                            )
                        th = tpool.tile([128, nblk], F16, tag="th")
                        nc.scalar.activation(
                            th[:],
                            pt[:],
                            mybir.ActivationFunctionType.Tanh,
                            bias=wh_sb[:, m * BLOC + b : m * BLOC + b + 1],
                        )
                        if vdot == "pe":
                            nc.tensor.matmul(
                                sc[:],
                                v_sb[:, m : m + 1],
                                th[:],
                                start=(m == 0),
                                stop=(m == MC - 1),
                            )
                        else:
                            # acc = th * v[m] (+ acc)  on the vector engine
                            if m == 0:
                                acc = tpool.tile([128, nblk], F16, tag="acc")
                                nc.vector.tensor_scalar_mul(
                                    acc[:], th[:], v32_sb[:, m : m + 1]
                                )
                            else:
                                nc.vector.scalar_tensor_tensor(
                                    acc[:],
                                    th[:],
                                    v32_sb[:, m : m + 1],
                                    acc[:],
                                    op0=mybir.AluOpType.mult,
                                    op1=mybir.AluOpType.add,
                                )
                    if vdot == "dve":
                        nc.tensor.matmul(
                            sc[:], ones_sb[:], acc[:], start=True, stop=True
                        )
                    nc.scalar.copy(
                        strip[:, g0 : g0 + nblk], sc[:]
                    )

                    if sb == sblk - 1:
                        # batch row b complete: run its softmax now (on the
                        # partition-0 strip) so only the last row's softmax
                        # sits in the kernel tail
                        seg = slice(b * S, (b + 1) * S)
                        bc = slice(b, b + 1)
                        nc.vector.tensor_add(
                            strip[:, seg], strip[:, seg], mask_sb[:, seg]
                        )
                        nc.vector.reduce_max(
                            negmax[:, bc], strip[:, seg],
                            axis=mybir.AxisListType.X, negate=True,
                        )
                        nc.scalar.activation(
                            strip[:, seg],
                            strip[:, seg],
                            mybir.ActivationFunctionType.Exp,
                            bias=negmax[:, bc],
                            accum_out=sumexp[:, bc],
                        )
                        nc.vector.reciprocal(rsum[:, bc], sumexp[:, bc])
                        nc.vector.tensor_scalar_mul(
                            strip[:, seg], strip[:, seg], rsum[:, bc]
                        )
                        nc.sync.dma_start(out[b : b + 1, :], strip[:, seg])

    nc.compile()
    return nc


_NC = None


def _get_program():
    global _NC
    if _NC is None:
        _NC = _build_program()
    return _NC


def _prepare_in_maps(inputs, hidden, pad_matrix, W, U, v):
    inputs = np.asarray(inputs, dtype=np.float32)
    hidden = np.asarray(hidden, dtype=np.float32)
    pad_matrix = np.asarray(pad_matrix)
    W = np.asarray(W, dtype=np.float32)
    U = np.asarray(U, dtype=np.float32)
    v = np.asarray(v, dtype=np.float32)

    # xT_all[h, b, s] = inputs[s, b, h]
    xt_all = np.ascontiguousarray(inputs.transpose(2, 1, 0)).astype(np.float16)
    # U tiled: ut[p, ((k*MC + m)*128 + j)] = U[k*128+p, m*128+j]
    ut = np.ascontiguousarray(
        U.reshape(KC, 128, MC, 128).transpose(1, 0, 2, 3)
    ).reshape(128, KC * MC * 128).astype(np.float16)
    # bias Wh = hidden[0] @ W, fp32 on host (0.05% of total FLOPs)
    Wh = hidden[0] @ W  # [B, H]
    # v tiled: vv[p, m] = v[m*128+p]
    vvf = np.ascontiguousarray(v[:, 0].reshape(MC, 128).T).astype(np.float32)
    vv = vvf.astype(np.float16)

    in_maps = []
    for c in range(NCORES):
        b0 = c * BLOC
        xt_c = np.ascontiguousarray(
            xt_all[:, b0 : b0 + BLOC, :].reshape(H, ROWS)
        )
        # wh[p, m*BLOC + b] = Wh[b0+b, m*128+p]
        wh_c = np.ascontiguousarray(
            Wh[b0 : b0 + BLOC].reshape(BLOC, MC, 128).transpose(2, 1, 0)
        ).reshape(128, MC * BLOC).astype(np.float32)
        mask_c = np.where(pad_matrix[b0 : b0 + BLOC], -100000.0, 0.0).astype(
            np.float32
        )
        in_maps.append(
            {"xt": xt_c, "ut": ut, "wh": wh_c, "vv": vv, "vvf": vvf,
             "mask": mask_c}
        )
    return in_maps


def kernel(inputs, hidden, pad_matrix, W, U, v):
    nc = _get_program()
    in_maps = _prepare_in_maps(inputs, hidden, pad_matrix, W, U, v)
    res = run_bass_kernel_spmd(nc, in_maps, core_ids=list(range(NCORES)))
    out = np.concatenate([r["out"] for r in res.results], axis=0)
    return out.astype(np.float32)


# revision 36
# speedup vs baseline: 1.3936x; 1.0037x over previous
"""Trainium2 Bass kernel for nn_Attention_3298534884255.

Computes, for inputs x:[S,B,H], hidden:[1,B,H], pad:[B,S], W,U:[H,H], v:[H,1]:
    scores[s,b] = v . tanh(hidden[0]@W [b] + (x[s,b] @ U))
    out = softmax(where(pad, -1e5, scores.T), axis=1)   -> [B, S]

Strategy: pure data parallelism over batch B=64 across 8 NeuronCores (8 batch
rows per core). W/U/v are tiny and replicated; the heavy op is x@U
(2*S*B*H^2 = 275 GFLOP total). Per core the matmul is computed in a
"proj-transposed" layout: psum[h_out, row] = sum_k U[k,h_out] * xT[k,row],
so U's natural layout is the stationary operand and xT (host-pretransposed)
streams. The Wh bias is per-partition in this layout (rows of one block share
one batch), so it fuses into the scalar-engine tanh. The v-dot runs on the
vector engine (scalar_tensor_tensor chain over the 8 h_out chunks) plus one
ones-vector matmul for the final partition reduction, keeping the PE free for
the main matmul stream (~220ns per 128x128x512 fp16 matmul = streaming
roofline). Per-batch softmax overlaps the remaining compute; only the last
row's softmax sits in the kernel tail.

S, B, H = 2048, 64, 1024. fp16 operands into the PE (fp32 accumulation).
"""

import sys

import numpy as np

if "/opt/trn_rl_repo" not in sys.path:
    sys.path.insert(0, "/opt/trn_rl_repo")

import concourse.tile as tile
from concourse import bacc, mybir
from concourse.bass_utils import run_bass_kernel_spmd

S, B, H = 2048, 64, 1024
NCORES = 8
BLOC = B // NCORES          # batch rows per core = 8
ROWS = S * BLOC             # rows per core = 16384
NBLK = 512                  # rows per block (one PSUM bank of fp32)
SBLK = S // NBLK            # s-blocks per batch row = 4
KC = H // 128               # contraction chunks = 8
MC = H // 128               # h_out chunks = 8

F16 = mybir.dt.float16
F32 = mybir.dt.float32


def _build_program(reps=1, vdot="dve", nblk=NBLK):
    sblk = S // nblk
    nc = bacc.Bacc(
        "TRN2", target_bir_lowering=False, debug=False, num_devices=NCORES
    )

    xt = nc.dram_tensor("xt", [H, ROWS], F16, kind="ExternalInput").ap()
    ut = nc.dram_tensor("ut", [128, KC * MC * 128], F16, kind="ExternalInput").ap()
    wh = nc.dram_tensor("wh", [128, MC * BLOC], F32, kind="ExternalInput").ap()
    vv = nc.dram_tensor("vv", [128, MC], F16, kind="ExternalInput").ap()
    vvf = nc.dram_tensor("vvf", [128, MC], F32, kind="ExternalInput").ap()
    mask = nc.dram_tensor("mask", [BLOC, S], F32, kind="ExternalInput").ap()
    out = nc.dram_tensor("out", [BLOC, S], F32, kind="ExternalOutput").ap()

    with tile.TileContext(nc) as tc:
        with (
            tc.tile_pool(name="consts", bufs=1) as consts,
            tc.tile_pool(name="xblk", bufs=3) as xpool,
            tc.tile_pool(name="tanh", bufs=4) as tpool,
            tc.tile_pool(
                name="proj_ps", bufs=(3 if nblk <= 512 else 2), space="PSUM"
            ) as pspool,
            tc.tile_pool(name="score_ps", bufs=2, space="PSUM") as scpool,
            tc.tile_pool(name="softmax", bufs=1) as smpool,
        ):
            # U arrives in per-k-chunk DMAs so the first block's matmuls can
            # start as soon as chunk 0 + the first x block land; small consts
            # ride the gpsimd (SWDGE) queue in parallel with the sync queue.
            u_sb = consts.tile([128, KC * MC * 128], F16)
            u_rest_loaded = [False]
            ucw = MC * 128
            nc.sync.dma_start(u_sb[:, 0:ucw], ut[:, 0:ucw])
            wh_sb = consts.tile([128, MC * BLOC], F32)
            nc.gpsimd.dma_start(wh_sb[:], wh[:])
            v_sb = consts.tile([128, MC], F16)
            nc.gpsimd.dma_start(v_sb[:], vv[:])
            v32_sb = consts.tile([128, MC], F32)
            nc.gpsimd.dma_start(v32_sb[:], vvf[:])
            ones_sb = consts.tile([128, 1], F16)
            nc.vector.memset(ones_sb[:], 1.0)
            mask_sb = consts.tile([1, ROWS], F32)
            nc.gpsimd.dma_start(mask_sb[:], mask[:])
            strip = consts.tile([1, ROWS], F32)

            # warm the PE clock (HAM) with throwaway matmuls while the first
            # real operands are still in flight on the DMA queues
            warm_sb = consts.tile([128, 512], F16)
            nc.vector.memset(warm_sb[:], 0.0)
            warm_ps = pspool.tile([128, 512], F32, tag="pt")
            for _ in range(64):
                nc.tensor.matmul(
                    warm_ps[:, 0:128], warm_sb[:, 0:128], warm_sb[:, 0:128],
                    start=True, stop=True,
                )

            xt_r = xt.rearrange("(k p) n -> p k n", p=128)

            negmax = smpool.tile([1, BLOC], F32, tag="negmax")
            cmax = smpool.tile([1, BLOC * SBLK], F32, tag="cmax")
            sumexp = smpool.tile([1, BLOC], F32, tag="sumexp")
            rsum = smpool.tile([1, BLOC], F32, tag="rsum")

            for b, sb in [
                (b, sb)
                for _ in range(reps)
                for b in range(BLOC)
                for sb in range(sblk)
            ]:
                    g0 = b * S + sb * nblk
                    xb = xpool.tile([128, KC * nblk], F16, tag="xb")
                    if (b, sb) == (0, 0):
                        for kk in range(KC):
                            nc.sync.dma_start(
                                xb[:, kk * nblk : (kk + 1) * nblk],
                                xt_r[:, kk, g0 : g0 + nblk],
                            )
                    else:
                        nc.sync.dma_start(
                            xb[:].rearrange("p (k n) -> p k n", k=KC),
                            xt_r[:, :, g0 : g0 + nblk],
                        )
                    if not u_rest_loaded[0]:
                        # remaining U chunks, behind the first x block
                        u_rest_loaded[0] = True
                        for kk in range(1, KC):
                            nc.sync.dma_start(
                                u_sb[:, kk * ucw : (kk + 1) * ucw],
                                ut[:, kk * ucw : (kk + 1) * ucw],
                            )
                    sc = scpool.tile([1, nblk], F32, tag="sc")
                    acc = None
                    vd = vdot
                    for m in range(MC):
                        pt = pspool.tile([128, nblk], F32, tag="pt")
                        for k in range(KC):
                            nc.tensor.matmul(
                                pt[:],
                                u_sb[:, (k * MC + m) * 128 : (k * MC + m + 1) * 128],
                                xb[:, k * nblk : (k + 1) * nblk],
                                start=(k == 0),
                                stop=(k == KC - 1),
                            )
                        th = tpool.tile([128, nblk], F16, tag="th")
                        nc.scalar.activation(
                            th[:],
                            pt[:],
                            mybir.ActivationFunctionType.Tanh,
                            bias=wh_sb[:, m * BLOC + b : m * BLOC + b + 1],
                        )
                        if vd == "pe":
                            nc.tensor.matmul(
                                sc[:],
                                v_sb[:, m : m + 1],
                                th[:],
                                start=(m == 0),
                                stop=(m == MC - 1),
                            )
                        else:
                            # acc = th * v[m] (+ acc)  on the vector engine
                            if m == 0:
                                acc = tpool.tile([128, nblk], F16, tag="acc")
                                nc.vector.tensor_scalar_mul(
                                    acc[:], th[:], v32_sb[:, m : m + 1]
                                )
                            else:
                                nc.vector.scalar_tensor_tensor(
                                    acc[:],
                                    th[:],
                                    v32_sb[:, m : m + 1],
                                    acc[:],
                                    op0=mybir.AluOpType.mult,
                                    op1=mybir.AluOpType.add,
                                )
                    if vd == "dve":
                        nc.tensor.matmul(
                            sc[:], ones_sb[:], acc[:], start=True, stop=True
                        )
                    # move scores out of PSUM fused with the additive mask,
                    # and track the chunk max so the end-of-row softmax only
                    # needs exp + normalize
                    nc.vector.tensor_add(
                        strip[:, g0 : g0 + nblk],
                        sc[:],
                        mask_sb[:, g0 : g0 + nblk],
                    )
                    ci = b * sblk + sb
                    nc.vector.reduce_max(
                        cmax[:, ci : ci + 1],
                        strip[:, g0 : g0 + nblk],
                        axis=mybir.AxisListType.X,
                    )

                    if sb == sblk - 1:
                        # batch row b complete: run its softmax now (on the
                        # partition-0 strip) so only the last row's softmax
                        # sits in the kernel tail
                        seg = slice(b * S, (b + 1) * S)
                        bc = slice(b, b + 1)
                        nc.vector.reduce_max(
                            negmax[:, bc],
                            cmax[:, b * sblk : (b + 1) * sblk],
                            axis=mybir.AxisListType.X, negate=True,
                        )
                        nc.scalar.activation(
                            strip[:, seg],
                            strip[:, seg],
                            mybir.ActivationFunctionType.Exp,
                            bias=negmax[:, bc],
                            accum_out=sumexp[:, bc],
                        )
                        nc.vector.reciprocal(rsum[:, bc], sumexp[:, bc])
                        nc.vector.tensor_scalar_mul(
                            strip[:, seg], strip[:, seg], rsum[:, bc]
                        )
                        nc.sync.dma_start(out[b : b + 1, :], strip[:, seg])

    nc.compile()
    return nc


_NC = None


def _get_program():
    global _NC
    if _NC is None:
        _NC = _build_program()
    return _NC


def _prepare_in_maps(inputs, hidden, pad_matrix, W, U, v):
    inputs = np.asarray(inputs, dtype=np.float32)
    hidden = np.asarray(hidden, dtype=np.float32)
    pad_matrix = np.asarray(pad_matrix)
    W = np.asarray(W, dtype=np.float32)
    U = np.asarray(U, dtype=np.float32)
    v = np.asarray(v, dtype=np.float32)

    # xT_all[h, b, s] = inputs[s, b, h]
    xt_all = np.ascontiguousarray(inputs.transpose(2, 1, 0)).astype(np.float16)
    # U tiled: ut[p, ((k*MC + m)*128 + j)] = U[k*128+p, m*128+j]
    ut = np.ascontiguousarray(
        U.reshape(KC, 128, MC, 128).transpose(1, 0, 2, 3)
    ).reshape(128, KC * MC * 128).astype(np.float16)
    # bias Wh = hidden[0] @ W, fp32 on host (0.05% of total FLOPs)
    Wh = hidden[0] @ W  # [B, H]
    # v tiled: vv[p, m] = v[m*128+p]
    vvf = np.ascontiguousarray(v[:, 0].reshape(MC, 128).T).astype(np.float32)
    vv = vvf.astype(np.float16)

    in_maps = []
    for c in range(NCORES):
        b0 = c * BLOC
        xt_c = np.ascontiguousarray(
            xt_all[:, b0 : b0 + BLOC, :].reshape(H, ROWS)
        )
        # wh[p, m*BLOC + b] = Wh[b0+b, m*128+p]
        wh_c = np.ascontiguousarray(
            Wh[b0 : b0 + BLOC].reshape(BLOC, MC, 128).transpose(2, 1, 0)
        ).reshape(128, MC * BLOC).astype(np.float32)
        mask_c = np.where(pad_matrix[b0 : b0 + BLOC], -100000.0, 0.0).astype(
            np.float32
        )
        in_maps.append(
            {"xt": xt_c, "ut": ut, "wh": wh_c, "vv": vv, "vvf": vvf,
             "mask": mask_c}
        )
    return in_maps


def kernel(inputs, hidden, pad_matrix, W, U, v):
    nc = _get_program()
    in_maps = _prepare_in_maps(inputs, hidden, pad_matrix, W, U, v)
    res = run_bass_kernel_spmd(nc, in_maps, core_ids=list(range(NCORES)))
    out = np.concatenate([r["out"] for r in res.results], axis=0)
    return out.astype(np.float32)


# revision 39
# speedup vs baseline: 1.3999x; 1.0045x over previous
"""Trainium2 Bass kernel for nn_Attention_3298534884255.

Computes, for inputs x:[S,B,H], hidden:[1,B,H], pad:[B,S], W,U:[H,H], v:[H,1]:
    scores[s,b] = v . tanh(hidden[0]@W [b] + (x[s,b] @ U))
    out = softmax(where(pad, -1e5, scores.T), axis=1)   -> [B, S]

Strategy: pure data parallelism over batch B=64 across 8 NeuronCores (8 batch
rows per core). W/U/v are tiny and replicated; the heavy op is x@U
(2*S*B*H^2 = 275 GFLOP total). Per core the matmul is computed in a
"proj-transposed" layout: psum[h_out, row] = sum_k U[k,h_out] * xT[k,row],
so U's natural layout is the stationary operand and xT (host-pretransposed)
streams. The Wh bias is per-partition in this layout (rows of one block share
one batch), so it fuses into the scalar-engine tanh. The v-dot runs on the
vector engine (scalar_tensor_tensor chain over the 8 h_out chunks) plus one
ones-vector matmul for the final partition reduction, keeping the PE free for
the main matmul stream (~220ns per 128x128x512 fp16 matmul = streaming
roofline). Per-batch softmax overlaps the remaining compute; only the last
row's softmax sits in the kernel tail.

S, B, H = 2048, 64, 1024. fp16 operands into the PE (fp32 accumulation).
"""

import sys

import numpy as np

if "/opt/trn_rl_repo" not in sys.path:
    sys.path.insert(0, "/opt/trn_rl_repo")

import concourse.tile as tile
from concourse import bacc, bass_isa, mybir
from concourse.bass_utils import run_bass_kernel_spmd

S, B, H = 2048, 64, 1024
NCORES = 8
BLOC = B // NCORES          # batch rows per core = 8
ROWS = S * BLOC             # rows per core = 16384
NBLK = 512                  # rows per block (one PSUM bank of fp32)
SBLK = S // NBLK            # s-blocks per batch row = 4
KC = H // 128               # contraction chunks = 8
MC = H // 128               # h_out chunks = 8

F16 = mybir.dt.float16
F32 = mybir.dt.float32


def _build_program(reps=1, vdot="dve", nblk=NBLK):
    sblk = S // nblk
    nc = bacc.Bacc(
        "TRN2", target_bir_lowering=False, debug=False, num_devices=NCORES
    )

    xt = nc.dram_tensor("xt", [H, ROWS], F16, kind="ExternalInput").ap()
    ut = nc.dram_tensor("ut", [128, KC * MC * 128], F16, kind="ExternalInput").ap()
    wh = nc.dram_tensor("wh", [128, MC * BLOC], F32, kind="ExternalInput").ap()
    vv = nc.dram_tensor("vv", [128, MC], F16, kind="ExternalInput").ap()
    vvf = nc.dram_tensor("vvf", [128, MC], F32, kind="ExternalInput").ap()
    mask = nc.dram_tensor("mask", [BLOC, S], F32, kind="ExternalInput").ap()
    out = nc.dram_tensor("out", [BLOC, S], F32, kind="ExternalOutput").ap()

    with tile.TileContext(nc) as tc:
        with (
            tc.tile_pool(name="consts", bufs=1) as consts,
            tc.tile_pool(name="xblk", bufs=3) as xpool,
            tc.tile_pool(name="tanh", bufs=4) as tpool,
            tc.tile_pool(
                name="proj_ps", bufs=(3 if nblk <= 512 else 2), space="PSUM"
            ) as pspool,
            tc.tile_pool(name="score_ps", bufs=2, space="PSUM") as scpool,
            tc.tile_pool(name="softmax", bufs=1) as smpool,
        ):
            # U arrives in per-k-chunk DMAs so the first block's matmuls can
            # start as soon as chunk 0 + the first x block land; small consts
            # ride the gpsimd (SWDGE) queue in parallel with the sync queue.
            u_sb = consts.tile([128, KC * MC * 128], F16)
            u_rest_loaded = [False]
            ucw = MC * 128
            nc.sync.dma_start(u_sb[:, 0:ucw], ut[:, 0:ucw])
            wh_sb = consts.tile([128, MC * BLOC], F32)
            nc.gpsimd.dma_start(wh_sb[:], wh[:])
            v_sb = consts.tile([128, MC], F16)
            nc.gpsimd.dma_start(v_sb[:], vv[:])
            v32_sb = consts.tile([128, MC], F32)
            nc.gpsimd.dma_start(v32_sb[:], vvf[:])
            ones_sb = consts.tile([128, 1], F16)
            nc.vector.memset(ones_sb[:], 1.0)
            mask_sb = consts.tile([1, ROWS], F32)
            nc.gpsimd.dma_start(mask_sb[:], mask[:])
            strip = consts.tile([1, ROWS], F32)

            # warm the PE clock (HAM) with throwaway matmuls while the first
            # real operands are still in flight on the DMA queues
            warm_sb = consts.tile([128, 512], F16)
            nc.vector.memset(warm_sb[:], 0.0)
            warm_ps = pspool.tile([128, 512], F32, tag="pt")
            for _ in range(64):
                nc.tensor.matmul(
                    warm_ps[:, 0:128], warm_sb[:, 0:128], warm_sb[:, 0:128],
                    start=True, stop=True,
                )

            xt_r = xt.rearrange("(k p) n -> p k n", p=128)

            negmax = smpool.tile([1, BLOC], F32, tag="negmax")
            cmax = smpool.tile([1, BLOC * SBLK], F32, tag="cmax")
            sumexp = smpool.tile([1, BLOC], F32, tag="sumexp")
            rsum = smpool.tile([1, BLOC], F32, tag="rsum")

            for b, sb in [
                (b, sb)
                for _ in range(reps)
                for b in range(BLOC)
                for sb in range(sblk)
            ]:
                    g0 = b * S + sb * nblk
                    xb = xpool.tile([128, KC * nblk], F16, tag="xb")
                    if (b, sb) == (0, 0):
                        for kk in range(KC):
                            nc.sync.dma_start(
                                xb[:, kk * nblk : (kk + 1) * nblk],
                                xt_r[:, kk, g0 : g0 + nblk],
                            )
                    else:
                        nc.sync.dma_start(
                            xb[:].rearrange("p (k n) -> p k n", k=KC),
                            xt_r[:, :, g0 : g0 + nblk],
                        )
                    if not u_rest_loaded[0]:
                        # remaining U chunks, behind the first x block
                        u_rest_loaded[0] = True
                        for kk in range(1, KC):
                            nc.sync.dma_start(
                                u_sb[:, kk * ucw : (kk + 1) * ucw],
                                ut[:, kk * ucw : (kk + 1) * ucw],
                            )
                    acc = None
                    vd = vdot
                    last = (b, sb) == (BLOC - 1, sblk - 1)
                    if vd == "pe" or last:
                        sc = scpool.tile([1, nblk], F32, tag="sc")
                    else:
                        sc = None
                    for m in range(MC):
                        pt = pspool.tile([128, nblk], F32, tag="pt")
                        for k in range(KC):
                            nc.tensor.matmul(
                                pt[:],
                                u_sb[:, (k * MC + m) * 128 : (k * MC + m + 1) * 128],
                                xb[:, k * nblk : (k + 1) * nblk],
                                start=(k == 0),
                                stop=(k == KC - 1),
                            )
                        th = tpool.tile([128, nblk], F16, tag="th")
                        nc.scalar.activation(
                            th[:],
                            pt[:],
                            mybir.ActivationFunctionType.Tanh,
                            bias=wh_sb[:, m * BLOC + b : m * BLOC + b + 1],
                        )
                        if vd == "pe":
                            nc.tensor.matmul(
                                sc[:],
                                v_sb[:, m : m + 1],
                                th[:],
                                start=(m == 0),
                                stop=(m == MC - 1),
                            )
                        else:
                            # acc = th * v[m] (+ acc)  on the vector engine
                            if m == 0:
                                acc = tpool.tile([128, nblk], F16, tag="acc")
                                nc.vector.tensor_scalar_mul(
                                    acc[:], th[:], v32_sb[:, m : m + 1]
                                )
                            else:
                                nc.vector.scalar_tensor_tensor(
                                    acc[:],
                                    th[:],
                                    v32_sb[:, m : m + 1],
                                    acc[:],
                                    op0=mybir.AluOpType.mult,
                                    op1=mybir.AluOpType.add,
                                )
                    if vd == "dve" and not last:
                        # final partition-sum on the (otherwise idle) GpSimd
                        red = tpool.tile([128, nblk], F32, tag="red")
                        nc.gpsimd.partition_all_reduce(
                            red[:], acc[:], 128, bass_isa.ReduceOp.add
                        )
                        score_row = red[0:1, :]
                    elif vd == "dve":
                        # last block: PE ones-matmul keeps the reduce off the
                        # kernel-tail critical path
                        nc.tensor.matmul(
                            sc[:], ones_sb[:], acc[:], start=True, stop=True
                        )
                        score_row = sc[:]
                    else:
                        score_row = sc[:]
                    # move scores to the strip fused with the additive mask,
                    # and track the chunk max so the end-of-row softmax only
                    # needs exp + normalize
                    nc.vector.tensor_add(
                        strip[:, g0 : g0 + nblk],
                        score_row,
                        mask_sb[:, g0 : g0 + nblk],
                    )
                    ci = b * sblk + sb
                    nc.vector.reduce_max(
                        cmax[:, ci : ci + 1],
                        strip[:, g0 : g0 + nblk],
                        axis=mybir.AxisListType.X,
                    )

                    if sb == sblk - 1:
                        # batch row b complete: run its softmax now (on the
                        # partition-0 strip) so only the last row's softmax
                        # sits in the kernel tail
                        seg = slice(b * S, (b + 1) * S)
                        bc = slice(b, b + 1)
                        nc.vector.reduce_max(
                            negmax[:, bc],
                            cmax[:, b * sblk : (b + 1) * sblk],
                            axis=mybir.AxisListType.X, negate=True,
                        )
                        nc.scalar.activation(
                            strip[:, seg],
                            strip[:, seg],
                            mybir.ActivationFunctionType.Exp,
                            bias=negmax[:, bc],
                            accum_out=sumexp[:, bc],
                        )
                        nc.vector.reciprocal(rsum[:, bc], sumexp[:, bc])
                        nc.vector.tensor_scalar_mul(
                            strip[:, seg], strip[:, seg], rsum[:, bc]
                        )
                        nc.sync.dma_start(out[b : b + 1, :], strip[:, seg])

    nc.compile()
    return nc


_NC = None


def _get_program():
    global _NC
    if _NC is None:
        _NC = _build_program()
    return _NC


def _prepare_in_maps(inputs, hidden, pad_matrix, W, U, v):
    inputs = np.asarray(inputs, dtype=np.float32)
    hidden = np.asarray(hidden, dtype=np.float32)
    pad_matrix = np.asarray(pad_matrix)
    W = np.asarray(W, dtype=np.float32)
    U = np.asarray(U, dtype=np.float32)
    v = np.asarray(v, dtype=np.float32)

    # xT_all[h, b, s] = inputs[s, b, h]
    xt_all = np.ascontiguousarray(inputs.transpose(2, 1, 0)).astype(np.float16)
    # U tiled: ut[p, ((k*MC + m)*128 + j)] = U[k*128+p, m*128+j]
    ut = np.ascontiguousarray(
        U.reshape(KC, 128, MC, 128).transpose(1, 0, 2, 3)
    ).reshape(128, KC * MC * 128).astype(np.float16)
    # bias Wh = hidden[0] @ W, fp32 on host (0.05% of total FLOPs)
    Wh = hidden[0] @ W  # [B, H]
    # v tiled: vv[p, m] = v[m*128+p]
    vvf = np.ascontiguousarray(v[:, 0].reshape(MC, 128).T).astype(np.float32)
    vv = vvf.astype(np.float16)

    in_maps = []
    for c in range(NCORES):
        b0 = c * BLOC
        xt_c = np.ascontiguousarray(
            xt_all[:, b0 : b0 + BLOC, :].reshape(H, ROWS)
        )
        # wh[p, m*BLOC + b] = Wh[b0+b, m*128+p]
        wh_c = np.ascontiguousarray(
            Wh[b0 : b0 + BLOC].reshape(BLOC, MC, 128).transpose(2, 1, 0)
        ).reshape(128, MC * BLOC).astype(np.float32)
        mask_c = np.where(pad_matrix[b0 : b0 + BLOC], -100000.0, 0.0).astype(
            np.float32
        )
        in_maps.append(
            {"xt": xt_c, "ut": ut, "wh": wh_c, "vv": vv, "vvf": vvf,
             "mask": mask_c}
        )
    return in_maps


def kernel(inputs, hidden, pad_matrix, W, U, v):
    nc = _get_program()
    in_maps = _prepare_in_maps(inputs, hidden, pad_matrix, W, U, v)
    res = run_bass_kernel_spmd(nc, in_maps, core_ids=list(range(NCORES)))
    out = np.concatenate([r["out"] for r in res.results], axis=0)
    return out.astype(np.float32)


# revision 40
# speedup vs baseline: 1.4302x; 1.0216x over previous
"""Trainium2 Bass kernel for nn_Attention_3298534884255.

Computes, for inputs x:[S,B,H], hidden:[1,B,H], pad:[B,S], W,U:[H,H], v:[H,1]:
    scores[s,b] = v . tanh(hidden[0]@W [b] + (x[s,b] @ U))
    out = softmax(where(pad, -1e5, scores.T), axis=1)   -> [B, S]

Strategy: pure data parallelism over batch B=64 across 8 NeuronCores (8 batch
rows per core). W/U/v are tiny and replicated; the heavy op is x@U
(2*S*B*H^2 = 275 GFLOP total). Per core the matmul is computed in a
"proj-transposed" layout: psum[h_out, row] = sum_k U[k,h_out] * xT[k,row],
so U's natural layout is the stationary operand and xT (host-pretransposed)
streams. The Wh bias is per-partition in this layout (rows of one block share
one batch), so it fuses into the scalar-engine tanh. The v-dot runs on the
vector engine (scalar_tensor_tensor chain over the 8 h_out chunks) plus one
ones-vector matmul for the final partition reduction, keeping the PE free for
the main matmul stream (~220ns per 128x128x512 fp16 matmul = streaming
roofline). Per-batch softmax overlaps the remaining compute; only the last
row's softmax sits in the kernel tail.

S, B, H = 2048, 64, 1024. fp16 operands into the PE (fp32 accumulation).
"""

import sys

import numpy as np

if "/opt/trn_rl_repo" not in sys.path:
    sys.path.insert(0, "/opt/trn_rl_repo")

import concourse.tile as tile
from concourse import bacc, bass_isa, mybir
from concourse.bass_utils import run_bass_kernel_spmd

S, B, H = 2048, 64, 1024
NCORES = 8
BLOC = B // NCORES          # batch rows per core = 8
ROWS = S * BLOC             # rows per core = 16384
NBLK = 512                  # rows per block (one PSUM bank of fp32)
SBLK = S // NBLK            # s-blocks per batch row = 4
KC = H // 128               # contraction chunks = 8
MC = H // 128               # h_out chunks = 8

F16 = mybir.dt.float16
F32 = mybir.dt.float32


def _build_program(reps=1, vdot="dve", nblk=NBLK):
    sblk = S // nblk
    nc = bacc.Bacc(
        "TRN2", target_bir_lowering=False, debug=False, num_devices=NCORES
    )

    xt = nc.dram_tensor("xt", [H, ROWS], F16, kind="ExternalInput").ap()
    ut = nc.dram_tensor("ut", [128, KC * MC * 128], F16, kind="ExternalInput").ap()
    wh = nc.dram_tensor("wh", [128, MC * BLOC], F32, kind="ExternalInput").ap()
    vv = nc.dram_tensor("vv", [128, MC], F16, kind="ExternalInput").ap()
    vvf = nc.dram_tensor("vvf", [128, MC], F32, kind="ExternalInput").ap()
    mask = nc.dram_tensor("mask", [BLOC, S], F32, kind="ExternalInput").ap()
    out = nc.dram_tensor("out", [BLOC, S], F32, kind="ExternalOutput").ap()

    with tile.TileContext(nc) as tc:
        with (
            tc.tile_pool(name="consts", bufs=1) as consts,
            tc.tile_pool(name="xblk", bufs=4) as xpool,
            tc.tile_pool(name="tanh", bufs=4) as tpool,
            tc.tile_pool(
                name="proj_ps", bufs=(4 if nblk <= 512 else 2), space="PSUM"
            ) as pspool,
            tc.tile_pool(name="score_ps", bufs=2, space="PSUM") as scpool,
            tc.tile_pool(name="softmax", bufs=1) as smpool,
        ):
            # U arrives in per-k-chunk DMAs so the first block's matmuls can
            # start as soon as chunk 0 + the first x block land; small consts
            # ride the gpsimd (SWDGE) queue in parallel with the sync queue.
            u_sb = consts.tile([128, KC * MC * 128], F16)
            u_rest_loaded = [False]
            ucw = MC * 128
            nc.sync.dma_start(u_sb[:, 0:ucw], ut[:, 0:ucw])
            wh_sb = consts.tile([128, MC * BLOC], F32)
            nc.gpsimd.dma_start(wh_sb[:], wh[:])
            v_sb = consts.tile([128, MC], F16)
            nc.gpsimd.dma_start(v_sb[:], vv[:])
            v32_sb = consts.tile([128, MC], F32)
            nc.gpsimd.dma_start(v32_sb[:], vvf[:])
            ones_sb = consts.tile([128, 1], F16)
            nc.vector.memset(ones_sb[:], 1.0)
            mask_sb = consts.tile([1, ROWS], F32)
            nc.gpsimd.dma_start(mask_sb[:], mask[:])
            strip = consts.tile([1, ROWS], F32)

            # warm the PE clock (HAM) with throwaway matmuls while the first
            # real operands are still in flight on the DMA queues
            warm_sb = consts.tile([128, 512], F16)
            nc.vector.memset(warm_sb[:], 0.0)
            warm_ps = pspool.tile([128, 512], F32, tag="pt")
            for _ in range(64):
                nc.tensor.matmul(
                    warm_ps[:, 0:128], warm_sb[:, 0:128], warm_sb[:, 0:128],
                    start=True, stop=True,
                )

            xt_r = xt.rearrange("(k p) n -> p k n", p=128)

            negmax = smpool.tile([1, BLOC], F32, tag="negmax")
            cmax = smpool.tile([1, BLOC * SBLK], F32, tag="cmax")
            sumexp = smpool.tile([1, BLOC], F32, tag="sumexp")
            rsum = smpool.tile([1, BLOC], F32, tag="rsum")

            for b, sb in [
                (b, sb)
                for _ in range(reps)
                for b in range(BLOC)
                for sb in range(sblk)
            ]:
                    g0 = b * S + sb * nblk
                    xb = xpool.tile([128, KC * nblk], F16, tag="xb")
                    if (b, sb) == (0, 0):
                        for kk in range(KC):
                            nc.sync.dma_start(
                                xb[:, kk * nblk : (kk + 1) * nblk],
                                xt_r[:, kk, g0 : g0 + nblk],
                            )
                    else:
                        nc.sync.dma_start(
                            xb[:].rearrange("p (k n) -> p k n", k=KC),
                            xt_r[:, :, g0 : g0 + nblk],
                        )
                    if not u_rest_loaded[0]:
                        # remaining U chunks, behind the first x block
                        u_rest_loaded[0] = True
                        for kk in range(1, KC):
                            nc.sync.dma_start(
                                u_sb[:, kk * ucw : (kk + 1) * ucw],
                                ut[:, kk * ucw : (kk + 1) * ucw],
                            )
                    acc = None
                    vd = vdot
                    last = (b, sb) == (BLOC - 1, sblk - 1)
                    if vd == "pe" or last:
                        sc = scpool.tile([1, nblk], F32, tag="sc")
                    else:
                        sc = None
                    for m in range(MC):
                        pt = pspool.tile([128, nblk], F32, tag="pt")
                        for k in range(KC):
                            nc.tensor.matmul(
                                pt[:],
                                u_sb[:, (k * MC + m) * 128 : (k * MC + m + 1) * 128],
                                xb[:, k * nblk : (k + 1) * nblk],
                                start=(k == 0),
                                stop=(k == KC - 1),
                            )
                        th = tpool.tile([128, nblk], F16, tag="th")
                        nc.scalar.activation(
                            th[:],
                            pt[:],
                            mybir.ActivationFunctionType.Tanh,
                            bias=wh_sb[:, m * BLOC + b : m * BLOC + b + 1],
                        )
                        if vd == "pe":
                            nc.tensor.matmul(
                                sc[:],
                                v_sb[:, m : m + 1],
                                th[:],
                                start=(m == 0),
                                stop=(m == MC - 1),
                            )
                        else:
                            # acc = th * v[m] (+ acc)  on the vector engine
                            if m == 0:
                                acc = tpool.tile([128, nblk], F16, tag="acc")
                                nc.vector.tensor_scalar_mul(
                                    acc[:], th[:], v32_sb[:, m : m + 1]
                                )
                            else:
                                nc.vector.scalar_tensor_tensor(
                                    acc[:],
                                    th[:],
                                    v32_sb[:, m : m + 1],
                                    acc[:],
                                    op0=mybir.AluOpType.mult,
                                    op1=mybir.AluOpType.add,
                                )
                    if vd == "dve" and not last:
                        # final partition-sum on the (otherwise idle) GpSimd
                        red = tpool.tile([128, nblk], F32, tag="red")
                        nc.gpsimd.partition_all_reduce(
                            red[:], acc[:], 128, bass_isa.ReduceOp.add
                        )
                        score_row = red[0:1, :]
                    elif vd == "dve":
                        # last block: PE ones-matmul keeps the reduce off the
                        # kernel-tail critical path
                        nc.tensor.matmul(
                            sc[:], ones_sb[:], acc[:], start=True, stop=True
                        )
                        score_row = sc[:]
                    else:
                        score_row = sc[:]
                    # move scores to the strip fused with the additive mask,
                    # and track the chunk max so the end-of-row softmax only
                    # needs exp + normalize
                    nc.vector.tensor_add(
                        strip[:, g0 : g0 + nblk],
                        score_row,
                        mask_sb[:, g0 : g0 + nblk],
                    )
                    ci = b * sblk + sb
                    nc.vector.reduce_max(
                        cmax[:, ci : ci + 1],
                        strip[:, g0 : g0 + nblk],
                        axis=mybir.AxisListType.X,
                    )

                    if sb == sblk - 1:
                        # batch row b complete: run its softmax now (on the
                        # partition-0 strip) so only the last row's softmax
                        # sits in the kernel tail
                        seg = slice(b * S, (b + 1) * S)
                        bc = slice(b, b + 1)
                        nc.vector.reduce_max(
                            negmax[:, bc],
                            cmax[:, b * sblk : (b + 1) * sblk],
                            axis=mybir.AxisListType.X, negate=True,
                        )
                        nc.scalar.activation(
                            strip[:, seg],
                            strip[:, seg],
                            mybir.ActivationFunctionType.Exp,
                            bias=negmax[:, bc],
                            accum_out=sumexp[:, bc],
                        )
                        nc.vector.reciprocal(rsum[:, bc], sumexp[:, bc])
                        nc.vector.tensor_scalar_mul(
                            strip[:, seg], strip[:, seg], rsum[:, bc]
                        )
                        nc.sync.dma_start(out[b : b + 1, :], strip[:, seg])

    nc.compile()
    return nc


_NC = None


def _get_program():
    global _NC
    if _NC is None:
        _NC = _build_program()
    return _NC


def _prepare_in_maps(inputs, hidden, pad_matrix, W, U, v):
    inputs = np.asarray(inputs, dtype=np.float32)
    hidden = np.asarray(hidden, dtype=np.float32)
    pad_matrix = np.asarray(pad_matrix)
    W = np.asarray(W, dtype=np.float32)
    U = np.asarray(U, dtype=np.float32)
    v = np.asarray(v, dtype=np.float32)

    # xT_all[h, b, s] = inputs[s, b, h]
    xt_all = np.ascontiguousarray(inputs.transpose(2, 1, 0)).astype(np.float16)
    # U tiled: ut[p, ((k*MC + m)*128 + j)] = U[k*128+p, m*128+j]
    ut = np.ascontiguousarray(
        U.reshape(KC, 128, MC, 128).transpose(1, 0, 2, 3)
    ).reshape(128, KC * MC * 128).astype(np.float16)
    # bias Wh = hidden[0] @ W, fp32 on host (0.05% of total FLOPs)
    Wh = hidden[0] @ W  # [B, H]
    # v tiled: vv[p, m] = v[m*128+p]
    vvf = np.ascontiguousarray(v[:, 0].reshape(MC, 128).T).astype(np.float32)
    vv = vvf.astype(np.float16)

    in_maps = []
    for c in range(NCORES):
        b0 = c * BLOC
        xt_c = np.ascontiguousarray(
            xt_all[:, b0 : b0 + BLOC, :].reshape(H, ROWS)
        )
        # wh[p, m*BLOC + b] = Wh[b0+b, m*128+p]
        wh_c = np.ascontiguousarray(
            Wh[b0 : b0 + BLOC].reshape(BLOC, MC, 128).transpose(2, 1, 0)
        ).reshape(128, MC * BLOC).astype(np.float32)
        mask_c = np.where(pad_matrix[b0 : b0 + BLOC], -100000.0, 0.0).astype(
            np.float32
        )
        in_maps.append(
            {"xt": xt_c, "ut": ut, "wh": wh_c, "vv": vv, "vvf": vvf,
             "mask": mask_c}
        )
    return in_maps


def kernel(inputs, hidden, pad_matrix, W, U, v):
    nc = _get_program()
    in_maps = _prepare_in_maps(inputs, hidden, pad_matrix, W, U, v)
    res = run_bass_kernel_spmd(nc, in_maps, core_ids=list(range(NCORES)))
    out = np.concatenate([r["out"] for r in res.results], axis=0)
    return out.astype(np.float32)


# revision 42
# speedup vs baseline: 1.4386x; 1.0058x over previous
"""Trainium2 Bass kernel for nn_Attention_3298534884255.

Computes, for inputs x:[S,B,H], hidden:[1,B,H], pad:[B,S], W,U:[H,H], v:[H,1]:
    scores[s,b] = v . tanh(hidden[0]@W [b] + (x[s,b] @ U))
    out = softmax(where(pad, -1e5, scores.T), axis=1)   -> [B, S]

Strategy: pure data parallelism over batch B=64 across 8 NeuronCores (8 batch
rows per core). W/U/v are tiny and replicated; the heavy op is x@U
(2*S*B*H^2 = 275 GFLOP total). Per core the matmul is computed in a
"proj-transposed" layout: psum[h_out, row] = sum_k U[k,h_out] * xT[k,row],
so U's natural layout is the stationary operand and xT (host-pretransposed)
streams. The Wh bias is per-partition in this layout (rows of one block share
one batch), so it fuses into the scalar-engine tanh. The v-dot runs on the
vector engine (scalar_tensor_tensor chain over the 8 h_out chunks) plus one
ones-vector matmul for the final partition reduction, keeping the PE free for
the main matmul stream (~220ns per 128x128x512 fp16 matmul = streaming
roofline). Per-batch softmax overlaps the remaining compute; only the last
row's softmax sits in the kernel tail.

S, B, H = 2048, 64, 1024. fp16 operands into the PE (fp32 accumulation).
"""

import sys

import numpy as np

if "/opt/trn_rl_repo" not in sys.path:
    sys.path.insert(0, "/opt/trn_rl_repo")

import concourse.tile as tile
from concourse import bacc, bass_isa, mybir
from concourse.bass_utils import run_bass_kernel_spmd

S, B, H = 2048, 64, 1024
NCORES = 8
BLOC = B // NCORES          # batch rows per core = 8
ROWS = S * BLOC             # rows per core = 16384
NBLK = 512                  # rows per block (one PSUM bank of fp32)
SBLK = S // NBLK            # s-blocks per batch row = 4
KC = H // 128               # contraction chunks = 8
MC = H // 128               # h_out chunks = 8

F16 = mybir.dt.float16
F32 = mybir.dt.float32


def _build_program(reps=1, vdot="dve", nblk=NBLK):
    sblk = S // nblk
    nc = bacc.Bacc(
        "TRN2", target_bir_lowering=False, debug=False, num_devices=NCORES
    )

    xt = nc.dram_tensor("xt", [H, ROWS], F16, kind="ExternalInput").ap()
    ut = nc.dram_tensor("ut", [128, KC * MC * 128], F16, kind="ExternalInput").ap()
    wh = nc.dram_tensor("wh", [128, MC * BLOC], F32, kind="ExternalInput").ap()
    vv = nc.dram_tensor("vv", [128, MC], F16, kind="ExternalInput").ap()
    vvf = nc.dram_tensor("vvf", [128, MC], F32, kind="ExternalInput").ap()
    mask = nc.dram_tensor("mask", [BLOC, S], F32, kind="ExternalInput").ap()
    out = nc.dram_tensor("out", [BLOC, S], F32, kind="ExternalOutput").ap()

    with tile.TileContext(nc) as tc:
        with (
            tc.tile_pool(name="consts", bufs=1) as consts,
            tc.tile_pool(name="xblk", bufs=4) as xpool,
            tc.tile_pool(name="tanh", bufs=4) as tpool,
            tc.tile_pool(
                name="proj_ps", bufs=(4 if nblk <= 512 else 2), space="PSUM"
            ) as pspool,
            tc.tile_pool(name="score_ps", bufs=2, space="PSUM") as scpool,
            tc.tile_pool(name="softmax", bufs=1) as smpool,
        ):
            # U arrives in per-k-chunk DMAs so the first block's matmuls can
            # start as soon as chunk 0 + the first x block land; small consts
            # ride the gpsimd (SWDGE) queue in parallel with the sync queue.
            u_sb = consts.tile([128, KC * MC * 128], F16)
            u_rest_loaded = [False]
            ucw = MC * 128
            nc.sync.dma_start(u_sb[:, 0:ucw], ut[:, 0:ucw])
            wh_sb = consts.tile([128, MC * BLOC], F32)
            nc.gpsimd.dma_start(wh_sb[:], wh[:])
            v_sb = consts.tile([128, MC], F16)
            nc.gpsimd.dma_start(v_sb[:], vv[:])
            v32_sb = consts.tile([128, MC], F32)
            nc.gpsimd.dma_start(v32_sb[:], vvf[:])
            ones_sb = consts.tile([128, 1], F16)
            nc.vector.memset(ones_sb[:], 1.0)
            mask_sb = consts.tile([1, ROWS], F32)
            nc.gpsimd.dma_start(mask_sb[:], mask[:])
            strip = consts.tile([1, ROWS], F32)

            # warm the PE clock (HAM) with throwaway matmuls while the first
            # real operands are still in flight on the DMA queues
            warm_sb = consts.tile([128, 512], F16)
            nc.vector.memset(warm_sb[:], 0.0)
            warm_ps = pspool.tile([128, 512], F32, tag="pt")
            for _ in range(64):
                nc.tensor.matmul(
                    warm_ps[:, 0:128], warm_sb[:, 0:128], warm_sb[:, 0:128],
                    start=True, stop=True,
                )

            xt_r = xt.rearrange("(k p) n -> p k n", p=128)

            negmax = smpool.tile([1, BLOC], F32, tag="negmax")
            cmax = smpool.tile([1, BLOC * SBLK], F32, tag="cmax")
            sumexp = smpool.tile([1, BLOC], F32, tag="sumexp")
            rsum = smpool.tile([1, BLOC], F32, tag="rsum")

            for b, sb in [
                (b, sb)
                for _ in range(reps)
                for b in range(BLOC)
                for sb in range(sblk)
            ]:
                    g0 = b * S + sb * nblk
                    xb = xpool.tile([128, KC * nblk], F16, tag="xb")
                    if (b, sb) == (0, 0):
                        for kk in range(KC):
                            nc.sync.dma_start(
                                xb[:, kk * nblk : (kk + 1) * nblk],
                                xt_r[:, kk, g0 : g0 + nblk],
                            )
                    else:
                        nc.sync.dma_start(
                            xb[:].rearrange("p (k n) -> p k n", k=KC),
                            xt_r[:, :, g0 : g0 + nblk],
                        )
                    if not u_rest_loaded[0]:
                        # remaining U chunks, behind the first x block
                        u_rest_loaded[0] = True
                        for kk in range(1, KC):
                            nc.sync.dma_start(
                                u_sb[:, kk * ucw : (kk + 1) * ucw],
                                ut[:, kk * ucw : (kk + 1) * ucw],
                            )
                    acc = None
                    vd = vdot
                    last = (b, sb) == (BLOC - 1, sblk - 1)
                    if vd == "pe" or last:
                        sc = scpool.tile([1, nblk], F32, tag="sc")
                    else:
                        sc = None
                    for m in range(MC):
                        pt = pspool.tile([128, nblk], F32, tag="pt")
                        for k in range(KC):
                            nc.tensor.matmul(
                                pt[:],
                                u_sb[:, (k * MC + m) * 128 : (k * MC + m + 1) * 128],
                                xb[:, k * nblk : (k + 1) * nblk],
                                start=(k == 0),
                                stop=(k == KC - 1),
                            )
                        th = tpool.tile([128, nblk], F16, tag="th")
                        nc.scalar.activation(
                            th[:],
                            pt[:],
                            mybir.ActivationFunctionType.Tanh,
                            bias=wh_sb[:, m * BLOC + b : m * BLOC + b + 1],
                        )
                        if vd == "pe":
                            nc.tensor.matmul(
                                sc[:],
                                v_sb[:, m : m + 1],
                                th[:],
                                start=(m == 0),
                                stop=(m == MC - 1),
                            )
                        else:
                            # acc = th * v[m] (+ acc)  on the vector engine
                            if m == 0:
                                acc = tpool.tile([128, nblk], F16, tag="acc")
                                nc.vector.tensor_scalar_mul(
                                    acc[:], th[:], v32_sb[:, m : m + 1]
                                )
                            else:
                                nc.vector.scalar_tensor_tensor(
                                    acc[:],
                                    th[:],
                                    v32_sb[:, m : m + 1],
                                    acc[:],
                                    op0=mybir.AluOpType.mult,
                                    op1=mybir.AluOpType.add,
                                )
                    if vd == "dve" and not last:
                        # final partition-sum on the (otherwise idle) GpSimd
                        red = tpool.tile([128, nblk], F32, tag="red")
                        nc.gpsimd.partition_all_reduce(
                            red[:], acc[:], 128, bass_isa.ReduceOp.add
                        )
                        score_row = red[0:1, :]
                    elif vd == "dve":
                        # last block: PE ones-matmul keeps the reduce off the
                        # kernel-tail critical path
                        nc.tensor.matmul(
                            sc[:], ones_sb[:], acc[:], start=True, stop=True
                        )
                        score_row = sc[:]
                    else:
                        score_row = sc[:]
                    # move scores to the strip fused with the additive mask,
                    # and track the chunk max so the end-of-row softmax only
                    # needs exp + normalize
                    nc.vector.tensor_add(
                        strip[:, g0 : g0 + nblk],
                        score_row,
                        mask_sb[:, g0 : g0 + nblk],
                    )
                    ci = b * sblk + sb
                    nc.vector.reduce_max(
                        cmax[:, ci : ci + 1],
                        strip[:, g0 : g0 + nblk],
                        axis=mybir.AxisListType.X,
                    )

                    if sb == sblk - 1:
                        # batch row b complete: run its softmax now (on the
                        # partition-0 strip) so only the last row's softmax
                        # sits in the kernel tail
                        seg = slice(b * S, (b + 1) * S)
                        bc = slice(b, b + 1)
                        nc.vector.reduce_max(
                            negmax[:, bc],
                            cmax[:, b * sblk : (b + 1) * sblk],
                            axis=mybir.AxisListType.X, negate=True,
                        )
                        nc.scalar.activation(
                            strip[:, seg],
                            strip[:, seg],
                            mybir.ActivationFunctionType.Exp,
                            bias=negmax[:, bc],
                            accum_out=sumexp[:, bc],
                        )
                        nc.vector.reciprocal(rsum[:, bc], sumexp[:, bc])
                        nc.vector.tensor_scalar_mul(
                            strip[:, seg], strip[:, seg], rsum[:, bc]
                        )
                        nc.sync.dma_start(out[b : b + 1, :], strip[:, seg])

    nc.compile()
    return nc


_NC = None


def _get_program():
    global _NC
    if _NC is None:
        _NC = _build_program()
    return _NC


def _prepare_in_maps(inputs, hidden, pad_matrix, W, U, v):
    inputs = np.asarray(inputs, dtype=np.float32)
    hidden = np.asarray(hidden, dtype=np.float32)
    pad_matrix = np.asarray(pad_matrix)
    W = np.asarray(W, dtype=np.float32)
    U = np.asarray(U, dtype=np.float32)
    v = np.asarray(v, dtype=np.float32)

    # xT_all[h, b, s] = inputs[s, b, h]
    xt_all = np.ascontiguousarray(inputs.transpose(2, 1, 0)).astype(np.float16)
    # U tiled: ut[p, ((k*MC + m)*128 + j)] = U[k*128+p, m*128+j]
    ut = np.ascontiguousarray(
        U.reshape(KC, 128, MC, 128).transpose(1, 0, 2, 3)
    ).reshape(128, KC * MC * 128).astype(np.float16)
    # bias Wh = hidden[0] @ W, fp32 on host (0.05% of total FLOPs)
    Wh = hidden[0] @ W  # [B, H]
    # v tiled: vv[p, m] = v[m*128+p]
    vvf = np.ascontiguousarray(v[:, 0].reshape(MC, 128).T).astype(np.float32)
    vv = vvf.astype(np.float16)

    in_maps = []
    for c in range(NCORES):
        b0 = c * BLOC
        xt_c = np.ascontiguousarray(
            xt_all[:, b0 : b0 + BLOC, :].reshape(H, ROWS)
        )
        # wh[p, m*BLOC + b] = Wh[b0+b, m*128+p]
        wh_c = np.ascontiguousarray(
            Wh[b0 : b0 + BLOC].reshape(BLOC, MC, 128).transpose(2, 1, 0)
        ).reshape(128, MC * BLOC).astype(np.float32)
        mask_c = np.where(pad_matrix[b0 : b0 + BLOC], -100000.0, 0.0).astype(
            np.float32
        )
        in_maps.append(
            {"xt": xt_c, "ut": ut, "wh": wh_c, "vv": vv, "vvf": vvf,
             "mask": mask_c}
        )
    return in_maps


def kernel(inputs, hidden, pad_matrix, W, U, v):
    nc = _get_program()
    in_maps = _prepare_in_maps(inputs, hidden, pad_matrix, W, U, v)
    res = run_bass_kernel_spmd(nc, in_maps, core_ids=list(range(NCORES)))
    out = np.concatenate([r["out"] for r in res.results], axis=0)
    return out.astype(np.float32)
